# revision 1
# baseline (speedup 1.0000x reference)
"""Channel-attention (transposed attention) Trainium2 Bass kernel.

Reference computation (per batch b of 8, one NeuronCore each):
    xt   = x[b].reshape(C, N).T                    # [N, C], N = 64*64 = 4096
    qkv  = xt @ w_qkv                              # [N, 3C]
    q, k, v : per-head [N, hd], nh=8, hd=64
    logits_h = k_h.T @ v_h                         # [hd, hd]
    attn_h   = softmax(scale * logits_h, axis=-1)  # scale = hd**-0.5 = 1/8
    out_h    = q_h @ attn_h.T                      # [N, hd]
    y[b] = (concat_h(out_h) @ w_proj + b_proj).T   # [C, N]

Sharding: data-parallel over batch, 1 batch item per core, no collectives.

On-core layout trick: x[b] is [C, N] in DRAM, which serves both operand
orientations of the QKV projection directly:
  - k/v with tokens on partitions:  lhsT = x chunk,  rhs = w_qkv cols
  - q^T with channels on partitions: lhsT = w_qkv cols, rhs = x chunk
so no activation transpose is ever needed. The per-head [64,64] softmax
matrices are transposed on the PE via an identity matmul, packed two heads
per 128 partitions as block-diagonal matrices so the second attention
matmul and the output projection run with full 128-row contraction.

The large (free-dim 512) matmuls use float32r (fp32 bytes, FP22
multiply) — 4x faster than true fp32 at free-dim >= 256, ~6e-5 relative
element precision. The small per-head logit matmuls (free-dim 64, where
f32r has no speed edge) stay exact fp32, which also permits the
tile_position col-64 packing that runs odd heads into partitions 64:128.
"""

import numpy as np

import concourse.bass as bass
import concourse.mybir as mybir
import concourse.tile as tile
from concourse import bass_utils

F32 = mybir.dt.float32
F32R = mybir.dt.float32r
AF = mybir.ActivationFunctionType

# Problem shape (hardcoded per contest contract).
B = 8
C = 512
H = W = 64
N = H * W            # 4096 tokens per batch
NH = 8               # heads
HD = C // NH         # 64
SCALE = HD ** -0.5   # 1/8
KC = C // 128        # 4 contraction chunks of 128 channels
NS = 8               # n-slices of 512 tokens
SL = N // NS         # 512
TT = SL // 128       # 4 token tiles of 128 per slice
HP = NH // 2         # 4 head pairs


def _r(ap):
    return ap.bitcast(F32R)


def _split_multi_waits(nc, max_waits=1):
    """The walrus build in this container encodes at most one sync-wait
    command per instruction (setupSyncWait raises "Too many sync wait
    commands" otherwise — the Tile kernel-tail drain carries several).
    Hoist excess waits onto same-engine NOPs immediately preceding the
    instruction; engine-FIFO order preserves the semantics."""
    n_split = 0
    for bb in nc.main_func.blocks:
        new_insts = []
        for ins in bb.instructions:
            si = ins.sync_info
            waits = list(si.on_wait) if si and si.on_wait else []
            if len(waits) > max_waits:
                extra, keep = waits[:-max_waits], waits[-max_waits:]
                while extra:
                    chunk, extra = extra[:max_waits], extra[max_waits:]
                    nop = mybir.InstNoOp(
                        name=nc.get_next_instruction_name(),
                        ins=[], outs=[],
                        engine=ins.engine,
                        sync_info=mybir.SyncInfo(on_wait=chunk, on_update=[]),
                    )
                    nc.register_instruction(nop)
                    new_insts.append(nop)
                    n_split += 1
                si.on_wait = keep
            new_insts.append(ins)
        bb.instructions[:] = new_insts
    return n_split


def build_nc(reps=1, phases='full'):
    nc = bass.Bass("TRN2", debug=False, num_devices=B)

    x_t = nc.dram_tensor("x", [C, N], F32, kind="ExternalInput")
    wq_t = nc.dram_tensor("w_qkv", [C, 3 * C], F32, kind="ExternalInput")
    wp_t = nc.dram_tensor("w_proj", [C, C], F32, kind="ExternalInput")
    bp_t = nc.dram_tensor("b_proj", [C, 1], F32, kind="ExternalInput")
    y_t = nc.dram_tensor("y", [C, N], F32, kind="ExternalOutput")
    id_t = nc.inline_tensor(np.eye(128, dtype=np.float32), name="id128")

    xd, wqd, wpd, bpd, yd = x_t.ap(), wq_t.ap(), wp_t.ap(), bp_t.ap(), y_t.ap()

    with tile.TileContext(nc) as tc:
        with (
            tc.tile_pool(name="const", bufs=1) as cpool,
            tc.tile_pool(name="qt", bufs=1) as qtpool,
            tc.tile_pool(name="soft", bufs=1) as spool,
        ):
            # ---- slice-0 activations first: the opening kv matmuls need
            # x[:, 0:512] + one wq section, so these 1MB of DMAs lead the
            # queue instead of trailing 4.2MB of weights ----
            x0_sb = [cpool.tile([128, SL], F32R, name=f"x0_{k}", tag=f"x0_{k}")
                     for k in range(KC)]
            for k in range(KC):
                nc.sync.dma_start(x0_sb[k][:], _r(xd[k * 128:(k + 1) * 128, 0:SL]))

            # ---- constants / weights resident in SBUF ----
            wq_sb = [cpool.tile([128, 3 * C], F32R, name=f"wq{k}", tag=f"wq{k}") for k in range(KC)]
            wp_sb = [cpool.tile([128, C], F32R, name=f"wp{k}", tag=f"wp{k}") for k in range(KC)]
            bp_sb = [cpool.tile([128, 1], F32, name=f"bp{k}", tag=f"bp{k}") for k in range(KC)]
            id_sb = cpool.tile([128, 128], F32, tag="id")
            for k in range(KC):
                r = slice(k * 128, (k + 1) * 128)
                for s in range(3):
                    cs = slice(s * C, (s + 1) * C)
                    nc.sync.dma_start(wq_sb[k][:, cs], _r(wqd[r, cs]))
            nc.sync.dma_start(id_sb[:], id_t.ap()[:, :])

            # q^T, resident for the whole batch: [C, N] as 4 chunks of 128 rows
            qt_sb = [qtpool.tile([128, N], F32R, name=f"qt{k}", tag=f"qt{k}") for k in range(KC)]
            for _rep in range(reps):
                _build_one_pass(nc, tc, spool, wq_sb, wp_sb, bp_sb, id_sb,
                                qt_sb, xd, yd, wpd, bpd, first_rep=(_rep == 0),
                                phases=phases, x0_sb=x0_sb)
    _split_multi_waits(nc)
    return nc


def _build_one_pass(nc, tc, spool, wq_sb, wp_sb, bp_sb, id_sb, qt_sb, xd, yd,
                    wpd, bpd, first_rep=True, phases="full", x0_sb=None):
    # phases: prefix gating for attribution benchmarks
    lvl = ["dma", "qkv", "logits", "soft", "attn", "full"].index(phases)
    if True:
        if True:

            # softmax logits accumulators: heads packed 2-per-128-partitions,
            # even heads (par=0) in lg_a partitions 0:64, odd heads (par=1)
            # in lg_b partitions 64:128 (separate banks so each partition
            # half runs its own clean psum accumulation group):
            # lg_{a,b}[64*par + d, hp*64 + e] = logits of head (2*hp + par)
            with tc.tile_pool(name="lgp", bufs=1, space="PSUM") as lgpool:
                lg_a = lgpool.tile([128, HP * HD], F32, tag="lg_a")
                lg_b = lgpool.tile([128, HP * HD], F32, tag="lg_b")
                lg = [lg_a, lg_b]

                # ================= Phase A: QKV + logit accumulation ========
                with (
                    tc.tile_pool(name="xin", bufs=3) as xpool,
                    tc.tile_pool(name="kvs", bufs=3) as kvpool,
                    tc.tile_pool(name="qps", bufs=2, space="PSUM") as qpsum,
                    tc.tile_pool(name="kvp", bufs=2, space="PSUM") as kvpsum,
                ):
                    for ns in range(NS):
                        nsl = slice(ns * SL, (ns + 1) * SL)
                        if first_rep and ns == 0 and x0_sb is not None:
                            xs = x0_sb  # preloaded ahead of the weights
                        else:
                            xs = [xpool.tile([128, SL], F32R, name=f"x{k}", tag=f"x{k}") for k in range(KC)]
                            for k in range(KC):
                                nc.sync.dma_start(
                                    xs[k][:], _r(xd[k * 128:(k + 1) * 128, nsl])
                                )
                        if lvl < 1:
                            continue
                        # --- k,v token tiles + logit accumulation ---
                        for t in range(TT):
                            kvp = kvpsum.tile([128, 2 * C], F32, tag="kv_ps")
                            for k in range(KC):
                                xk = xs[k][:, t * 128:(t + 1) * 128]
                                nc.tensor.matmul(
                                    kvp[:, 0:512], xk, wq_sb[k][:, 512:1024],
                                    start=(k == 0), stop=(k == KC - 1),
                                )
                                nc.tensor.matmul(
                                    kvp[:, 512:1024], xk, wq_sb[k][:, 1024:1536],
                                    start=(k == 0), stop=(k == KC - 1),
                                )
                            kv_sb = kvpool.tile([128, 2 * C], F32, tag="kv_sb")
                            nc.vector.tensor_copy(kv_sb[:], kvp[:])
                            if lvl < 2:
                                continue
                            first = ns == 0 and t == 0
                            last = ns == NS - 1 and t == TT - 1
                            for h in range(NH):
                                hp, par = divmod(h, 2)
                                # start=True marks the whole 2KB psum bank
                                # pending-zero, so only head 0/1 of the
                                # first tile starts each bank's group and
                                # only head 6/7 of the last tile stops it;
                                # the other heads' first write lands on
                                # still-pending bytes and overwrites.
                                nc.tensor.matmul(
                                    lg[par][par * 64:(par + 1) * 64,
                                            hp * 64:(hp + 1) * 64],
                                    kv_sb[:, h * 64:(h + 1) * 64],
                                    kv_sb[:, 512 + h * 64:512 + (h + 1) * 64],
                                    start=first and h < 2,
                                    stop=last and h >= NH - 2,
                                )

                        # --- q^T chunks: out[cq*128:, ns*512:] ---
                        for cq in range(KC):
                            qp = qpsum.tile([128, SL], F32, tag="qt_ps")
                            for k in range(KC):
                                nc.tensor.matmul(
                                    qp[:],
                                    wq_sb[k][:, cq * 128:(cq + 1) * 128],
                                    xs[k][:],
                                    start=(k == 0),
                                    stop=(k == KC - 1),
                                )
                            # ACT copy: keeps DVE free for the kv copies
                            # that gate the logit matmuls
                            nc.scalar.activation(qt_sb[cq][:, nsl], qp[:], AF.Copy)

                # deferred weight loads: w_proj/b_proj are first needed in
                # phase C/D, so their DMAs stay off the startup critical path
                if first_rep:
                    for k in range(KC):
                        r = slice(k * 128, (k + 1) * 128)
                        nc.sync.dma_start(wp_sb[k][:], _r(wpd[r, :]))
                        nc.sync.dma_start(bp_sb[k][:], bpd[r, :])

                if lvl < 3:
                    return
                # ================= Phase B: softmax + transposed attn =======
                # BD[hp]: block-diag exp(scale*(logits-max)) for head pair hp
                bd = [spool.tile([128, 128], F32, name=f"bd{p}", tag=f"bd{p}") for p in range(HP)]
                mx = spool.tile([128, HP], F32, tag="mx")
                bias = spool.tile([128, HP], F32, tag="bias")
                ssum = spool.tile([128, HP], F32, tag="ssum")
                recip = spool.tile([128, HP], F32, tag="recip")
                at_sb = [spool.tile([128, 128], F32R, name=f"at{p}", tag=f"at{p}") for p in range(HP)]

                for p in range(HP):
                    nc.gpsimd.memset(bd[p][:], 0.0)
                for p in range(HP):
                    csl = slice(p * 64, (p + 1) * 64)
                    for par in range(2):
                        psl = slice(par * 64, (par + 1) * 64)
                        nc.vector.reduce_max(
                            mx[psl, p:p + 1], lg[par][psl, csl],
                            axis=mybir.AxisListType.X,
                        )
                nc.vector.tensor_scalar_mul(bias[:], mx[:], -SCALE)
                for p in range(HP):
                    csl = slice(p * 64, (p + 1) * 64)
                    for par in range(2):
                        psl = slice(par * 64, (par + 1) * 64)
                        # diag block (par==0 -> cols 0:64, par==1 -> cols 64:128)
                        nc.scalar.activation(
                            bd[p][psl, psl], lg[par][psl, csl], AF.Exp,
                            bias=bias[psl, p:p + 1], scale=SCALE,
                        )
                        nc.vector.reduce_sum(
                            ssum[psl, p:p + 1], bd[p][psl, psl],
                            axis=mybir.AxisListType.X,
                        )
                nc.vector.reciprocal(recip[:], ssum[:])

            # transpose each block-diag exp matrix on the PE: at = bd^T
            with tc.tile_pool(name="bdt", bufs=2, space="PSUM") as bdtpool:
                for p in range(HP):
                    bdt = bdtpool.tile([128, 128], F32, tag="bdt")
                    nc.tensor.transpose(bdt[:], bd[p][:], id_sb[:])
                    nc.vector.tensor_copy(at_sb[p][:], bdt[:])

            if lvl < 4:
                return
            # ================= Phase C+D: attention apply + projection ======
            with (
                tc.tile_pool(name="ots", bufs=2) as otpool,
                tc.tile_pool(name="ys", bufs=3) as ypool,
                tc.tile_pool(name="otp", bufs=4, space="PSUM") as otpsum,
                tc.tile_pool(name="yp", bufs=3, space="PSUM") as ypsum,
            ):
                for ns in range(NS):
                    nsl = slice(ns * SL, (ns + 1) * SL)
                    ot_sb = []
                    for p in range(HP):
                        otp = otpsum.tile([128, SL], F32, tag="ot_ps")
                        nc.tensor.matmul(
                            otp[:], at_sb[p][:], qt_sb[p][:, nsl],
                            start=True, stop=True,
                        )
                        ot = otpool.tile([128, SL], F32R, name=f"ot{p}", tag=f"ot{p}")
                        # normalization: divide head-channel row d by sum_d
                        nc.vector.tensor_scalar_mul(ot[:], otp[:], recip[:, p:p + 1])
                        ot_sb.append(ot)
                    if lvl < 5:
                        continue
                    for co in range(KC):
                        yp = ypsum.tile([128, SL], F32, tag="y_ps")
                        for k in range(KC):
                            nc.tensor.matmul(
                                yp[:],
                                wp_sb[k][:, co * 128:(co + 1) * 128],
                                ot_sb[k][:],
                                start=(k == 0),
                                stop=(k == KC - 1),
                            )
                        ysb = ypool.tile([128, SL], F32, tag="y_sb")
                        nc.scalar.activation(
                            ysb[:], yp[:], AF.Identity,
                            bias=bp_sb[co][:, 0:1], scale=1.0,
                        )
                        nc.sync.dma_start(
                            yd[co * 128:(co + 1) * 128, nsl], ysb[:]
                        )


_NC_CACHE = None


def kernel(x, w_qkv, w_proj, b_proj, num_heads):
    x = np.ascontiguousarray(np.asarray(x, dtype=np.float32))
    w_qkv = np.ascontiguousarray(np.asarray(w_qkv, dtype=np.float32))
    w_proj = np.ascontiguousarray(np.asarray(w_proj, dtype=np.float32))
    b_proj = np.ascontiguousarray(np.asarray(b_proj, dtype=np.float32))
    assert int(num_heads) == NH
    assert x.shape == (B, C, H, W)

    xs = x.reshape(B, C, N)
    bp2 = b_proj.reshape(C, 1)
    in_maps = [
        {"x": xs[b], "w_qkv": w_qkv, "w_proj": w_proj, "b_proj": bp2}
        for b in range(B)
    ]
    global _NC_CACHE
    if _NC_CACHE is None:
        _NC_CACHE = build_nc()
    res = bass_utils.run_bass_kernel_spmd(_NC_CACHE, in_maps, list(range(B)))
    y = np.stack([res.results[b]["y"] for b in range(B)])
    return y.reshape(B, C, H, W).astype(np.float32)


if __name__ == "__main__":
    nc = build_nc()
    n_inst = sum(len(bb.instructions) for bb in nc.main_func.blocks)
    print(f"built OK, {n_inst} instructions")



# revision 12
# speedup vs baseline: 1.3012x; 1.3012x over previous
"""Channel-attention (transposed attention) Trainium2 Bass kernel.

Reference computation (per batch b of 8, one NeuronCore each):
    X    = x[b].reshape(C, N).T                    # [N, C], N = 64*64 = 4096
    qkv  = X @ w_qkv                               # [N, 3C]
    q, k, v : per-head [N, hd], nh=8, hd=64
    logits_h = k_h.T @ v_h                         # [hd, hd]
    attn_h   = softmax(scale * logits_h, axis=-1)  # scale = hd**-0.5 = 1/8
    out_h    = q_h @ attn_h.T                      # [N, hd]
    y[b] = (concat_h(out_h) @ w_proj + b_proj).T   # [C, N]

Sharding: data-parallel over batch, 1 batch item per core, no collectives.

Algebraic restructuring (the whole point of this kernel):

1. Gram trick. logits_h = k_h^T v_h = Wk_h^T (X^T X) Wv_h, so k and v are
   never materialized. G = X^T X is one [C, C] matmul with contraction
   over the 4096 tokens (PE-transposes of x feed it), then
   T = G @ Wv  ([C, C]) and lg = Wk^T T (pair-packed [128,128] blocks)
   are tiny. This kills the [N, 2C] kv projection and its PSUM->SBUF
   copies entirely.

2. Weight folding. out_h = q_h @ (E_h / s_h)^T with E = exp(scale*(lg-max)),
   s = rowsum(E), and y^T = w_proj^T out^T + b. Fold:
       y^T = Wy^T X^T,  Wy = Wq @ M^T,  M^T = blockdiag(E_h)^T-applied
       via  M^T[64h+e, :] = sum_d E_h[d,e] * (w_proj[64h+d, :] / s_h[d])
   M^T is 4 [128,512] matmuls (block-diag exp as lhsT, row-scaled w_proj
   as rhs), Wy is 16 more. This kills the q^T projection ([C, N]) and
   the attention-apply stage: phase C is a single [C, C] x [C, N] GEMM
   reading the resident x tiles directly.

All large matmuls are float32r (fp32 bytes, FP22 multiply): 1 PE
cycle/column at free-dim >= 256 vs 4 for fp32. PE transposes in f32r
run 1.5 cycles/row vs 2.0 for fp32. The softmax itself (reductions,
exp) is exact fp32.

Per-rep PE budget ~210k cycles vs ~396k for the direct formulation.
"""

import numpy as np

import concourse.bass as bass
import concourse.mybir as mybir
import concourse.tile as tile
from concourse import bass_utils

F32 = mybir.dt.float32
F32R = mybir.dt.float32r
AF = mybir.ActivationFunctionType
AX = mybir.AxisListType.X

# Problem shape (hardcoded per contest contract).
B = 8
C = 512
H = W = 64
N = H * W            # 4096 tokens per batch
NH = 8               # heads
HD = C // NH         # 64
SCALE = HD ** -0.5   # 1/8
KC = C // 128        # 4 chunks of 128 channels
NS = 8               # n-slices of 512 tokens
SL = N // NS         # 512
TT = SL // 128       # 4 token tiles of 128 per slice
HP = NH // 2         # 4 head pairs


def _r(ap):
    return ap.bitcast(F32R)


def _f(ap):
    return ap.bitcast(F32)


def _split_multi_waits(nc, max_waits=1):
    """The walrus build in this container encodes at most one sync-wait
    command per instruction (setupSyncWait raises "Too many sync wait
    commands" otherwise — the Tile kernel-tail drain carries several).
    Hoist excess waits onto same-engine NOPs immediately preceding the
    instruction; engine-FIFO order preserves the semantics."""
    n_split = 0
    for bb in nc.main_func.blocks:
        new_insts = []
        for ins in bb.instructions:
            si = ins.sync_info
            waits = list(si.on_wait) if si and si.on_wait else []
            if len(waits) > max_waits:
                extra, keep = waits[:-max_waits], waits[-max_waits:]
                while extra:
                    chunk, extra = extra[:max_waits], extra[max_waits:]
                    nop = mybir.InstNoOp(
                        name=nc.get_next_instruction_name(),
                        ins=[], outs=[],
                        engine=ins.engine,
                        sync_info=mybir.SyncInfo(on_wait=chunk, on_update=[]),
                    )
                    nc.register_instruction(nop)
                    new_insts.append(nop)
                    n_split += 1
                si.on_wait = keep
            new_insts.append(ins)
        bb.instructions[:] = new_insts
    return n_split


def build_nc(reps=1, phases='full'):
    nc = bass.Bass("TRN2", debug=False, num_devices=B)

    x_t = nc.dram_tensor("x", [C, N], F32, kind="ExternalInput")
    wq_t = nc.dram_tensor("w_qkv", [C, 3 * C], F32, kind="ExternalInput")
    wp_t = nc.dram_tensor("w_proj", [C, C], F32, kind="ExternalInput")
    bp_t = nc.dram_tensor("b_proj", [C, 1], F32, kind="ExternalInput")
    y_t = nc.dram_tensor("y", [C, N], F32, kind="ExternalOutput")
    id_t = nc.inline_tensor(np.eye(128, dtype=np.float32), name="id128")

    xd, wqd, wpd, bpd, yd = x_t.ap(), wq_t.ap(), wp_t.ap(), bp_t.ap(), y_t.ap()

    with tile.TileContext(nc) as tc:
        with (
            tc.tile_pool(name="const", bufs=1) as cpool,
            tc.tile_pool(name="xres", bufs=1) as xrpool,
        ):
            # resident x: [C, N] as 4 chunks x 8 slices of [128, 512].
            # slice-0 chunks lead the DMA queue (opening transposes need
            # them), weights follow, slices 1..7 stream per-slice.
            x_sb = [
                [xrpool.tile([128, SL], F32R, name=f"x{k}_{ns}", tag=f"x{k}_{ns}")
                 for ns in range(NS)]
                for k in range(KC)
            ]
            for k in range(KC):
                nc.sync.dma_start(x_sb[k][0][:], _r(xd[k * 128:(k + 1) * 128, 0:SL]))

            wq_sb = [cpool.tile([128, 3 * C], F32R, name=f"wq{k}", tag=f"wq{k}")
                     for k in range(KC)]
            id_sb = cpool.tile([128, 128], F32R, tag="id")
            for k in range(KC):
                r = slice(k * 128, (k + 1) * 128)
                for s in range(3):
                    cs = slice(s * C, (s + 1) * C)
                    nc.sync.dma_start(wq_sb[k][:, cs], _r(wqd[r, cs]))
            nc.sync.dma_start(id_sb[:], _r(id_t.ap()[:, :]))

            # w_proj (plain fp32: consumed by the DVE row-scale, not the PE)
            wp_sb = [cpool.tile([128, C], F32, name=f"wp{k}", tag=f"wp{k}")
                     for k in range(KC)]
            bp_sb = [cpool.tile([128, 1], F32, name=f"bp{k}", tag=f"bp{k}")
                     for k in range(KC)]

            # Wq^T, data-independent: 16 PE transposes at startup.
            wqT_sb = [cpool.tile([128, C], F32R, name=f"wqT{kq}", tag=f"wqT{kq}")
                      for kq in range(KC)]
            with tc.tile_pool(name="wtp", bufs=2, space="PSUM") as wtpool:
                for kq in range(KC):
                    tp = wtpool.tile([128, C], F32, tag="wqT_ps")
                    for kci in range(KC):
                        nc.tensor.matmul(
                            _r(tp[:, kci * 128:(kci + 1) * 128]),
                            wq_sb[kci][:, kq * 128:(kq + 1) * 128],
                            id_sb[:], is_transpose=True,
                            start=(kci == 0), stop=(kci == KC - 1),
                        )
                    nc.vector.tensor_copy(wqT_sb[kq][:], tp[:])

            for _rep in range(reps):
                _build_one_pass(nc, tc, cpool, wq_sb, wqT_sb, wp_sb, bp_sb,
                                id_sb, x_sb, xd, yd, wpd, bpd,
                                first_rep=(_rep == 0), phases=phases)
    _split_multi_waits(nc)
    return nc


def _build_one_pass(nc, tc, cpool, wq_sb, wqT_sb, wp_sb, bp_sb, id_sb, x_sb,
                    xd, yd, wpd, bpd, first_rep=True, phases="full"):
    lvl = ["dma", "gram", "logits", "soft", "wy", "full"].index(phases)

    # per-rep work tiles (stable tags: same storage every rep)
    G_sb = [cpool.tile([128, C], F32R, name=f"G{k}", tag=f"G{k}") for k in range(KC)]
    T_sb = [cpool.tile([128, C], F32R, name=f"T{k}", tag=f"T{k}") for k in range(KC)]
    mt_sb = [cpool.tile([128, C], F32R, name=f"mt{p}", tag=f"mt{p}") for p in range(HP)]
    wy_sb = [cpool.tile([128, C], F32R, name=f"wy{k}", tag=f"wy{k}") for k in range(KC)]
    wp2_sb = [cpool.tile([128, C], F32R, name=f"wp2{k}", tag=f"wp2{k}") for k in range(KC)]
    bd = [cpool.tile([128, 128], F32R, name=f"bd{p}", tag=f"bd{p}") for p in range(HP)]
    mx = cpool.tile([128, HP], F32, name="mx", tag="mx")
    bias = cpool.tile([128, HP], F32, name="bias", tag="bias")
    ssum = cpool.tile([128, HP], F32, name="ssum", tag="ssum")
    recip = cpool.tile([128, HP], F32, name="recip", tag="recip")

    # ================= Phase A: transpose x + Gram accumulation =========
    with (
        tc.tile_pool(name="xt", bufs=3) as xtpool,
        tc.tile_pool(name="gps", bufs=1, space="PSUM") as gpool,
        tc.tile_pool(name="tps", bufs=2, space="PSUM") as tpool,
    ):
        G_ps = [gpool.tile([128, C], F32, name=f"g_ps{k}", tag=f"g_ps{k}")
                for k in range(KC)]
        for ns in range(NS):
            if not (first_rep and ns == 0):
                nsl = slice(ns * SL, (ns + 1) * SL)
                for k in range(KC):
                    nc.sync.dma_start(
                        x_sb[k][ns][:], _r(xd[k * 128:(k + 1) * 128, nsl])
                    )
            if lvl < 1:
                continue
            for t in range(TT):
                tsl = slice(t * 128, (t + 1) * 128)
                tp = tpool.tile([128, C], F32, tag="xt_ps")
                for k in range(KC):
                    nc.tensor.matmul(
                        _r(tp[:, k * 128:(k + 1) * 128]),
                        x_sb[k][ns][:, tsl], id_sb[:], is_transpose=True,
                        start=(k == 0), stop=(k == KC - 1),
                    )
                xt = xtpool.tile([128, C], F32R, tag="xt_sb")
                # alternate copy engines so neither DVE nor ACT gates the PE
                if t % 2 == 0:
                    nc.vector.tensor_copy(xt[:], tp[:])
                else:
                    nc.scalar.activation(xt[:], tp[:], AF.Copy)
                for k in range(KC):
                    nc.tensor.matmul(
                        G_ps[k][:], xt[:, k * 128:(k + 1) * 128], xt[:],
                        start=(ns == 0 and t == 0),
                        stop=(ns == NS - 1 and t == TT - 1),
                    )
        if lvl >= 1:
            for k in range(KC):
                nc.vector.tensor_copy(G_sb[k][:], G_ps[k][:])

    # deferred weight loads: w_proj/b_proj first needed in phase B tail
    if first_rep:
        for k in range(KC):
            r = slice(k * 128, (k + 1) * 128)
            nc.sync.dma_start(wp_sb[k][:], wpd[r, :])
            nc.sync.dma_start(bp_sb[k][:], bpd[r, :])

    if lvl < 2:
        return
    # ================= Phase B: T = G Wv, logits, softmax, M^T, Wy ======
    with tc.tile_pool(name="Tps", bufs=1, space="PSUM") as Tpool:
        for kc in range(KC):
            Tp = Tpool.tile([128, C], F32, tag=f"T_ps{kc}")
            for k2 in range(KC):
                nc.tensor.matmul(
                    Tp[:], G_sb[k2][:, kc * 128:(kc + 1) * 128],
                    wq_sb[k2][:, 2 * C:3 * C],
                    start=(k2 == 0), stop=(k2 == KC - 1),
                )
            if kc % 2 == 0:
                nc.vector.tensor_copy(T_sb[kc][:], Tp[:])
            else:
                nc.scalar.activation(T_sb[kc][:], Tp[:], AF.Copy)

    with tc.tile_pool(name="lgp", bufs=1, space="PSUM") as lgpool:
        # lg bank [128, 512]: pair p occupies cols p*128, diag blocks are
        # per-head logits ([d,e] even head at [0:64,0:64] of the block,
        # odd at [64:128,64:128]); cross-head sub-blocks are junk and
        # never read. One start (pending-zeroes the bank), one stop.
        lg = lgpool.tile([128, HP * 128], F32, tag="lg")
        for p in range(HP):
            for kc in range(KC):
                nc.tensor.matmul(
                    lg[:, p * 128:(p + 1) * 128],
                    wq_sb[kc][:, C + p * 128:C + (p + 1) * 128],
                    T_sb[kc][:, p * 128:(p + 1) * 128],
                    start=(p == 0 and kc == 0),
                    stop=(p == HP - 1 and kc == KC - 1),
                )
        if lvl < 3:
            return
        # softmax over each diag block, exp into block-diag bd[p]
        if first_rep:
            for p in range(HP):
                nc.gpsimd.memset(_f(bd[p][:]), 0.0)
        for p in range(HP):
            for par in range(2):
                psl = slice(64 * par, 64 * par + 64)
                csl = slice(p * 128 + 64 * par, p * 128 + 64 * par + 64)
                nc.vector.reduce_max(mx[psl, p:p + 1], lg[psl, csl], axis=AX)
        nc.vector.tensor_scalar_mul(bias[:], mx[:], -SCALE)
        for p in range(HP):
            for par in range(2):
                psl = slice(64 * par, 64 * par + 64)
                csl = slice(p * 128 + 64 * par, p * 128 + 64 * par + 64)
                nc.scalar.activation(
                    bd[p][psl, psl], lg[psl, csl], AF.Exp,
                    bias=bias[psl, p:p + 1], scale=SCALE,
                )
                nc.vector.reduce_sum(ssum[psl, p:p + 1], _f(bd[p][psl, psl]), axis=AX)
        nc.vector.reciprocal(recip[:], ssum[:])

    if lvl < 4:
        return
    # w_proj rows pre-divided by the softmax row sums (fold of the
    # normalization): chunk p's partition layout matches recip[:, p]
    for p in range(HP):
        nc.vector.tensor_scalar_mul(wp2_sb[p][:], wp_sb[p][:], recip[:, p:p + 1])

    # M^T[128p + 64par + e, c] = sum_d E[d,e] * wp2[128p + 64par + d, c]
    with tc.tile_pool(name="mps", bufs=2, space="PSUM") as mpool:
        for p in range(HP):
            mp = mpool.tile([128, C], F32, tag="mt_ps")
            nc.tensor.matmul(mp[:], bd[p][:], wp2_sb[p][:],
                             start=True, stop=True)
            nc.vector.tensor_copy(mt_sb[p][:], mp[:])

    # Wy[ci, co] = sum_q Wq[ci, q] M^T[q, co]
    with tc.tile_pool(name="wyp", bufs=2, space="PSUM") as wypool:
        for ci in range(KC):
            wyp = wypool.tile([128, C], F32, tag="wy_ps")
            for kq in range(KC):
                nc.tensor.matmul(
                    wyp[:], wqT_sb[kq][:, ci * 128:(ci + 1) * 128], mt_sb[kq][:],
                    start=(kq == 0), stop=(kq == KC - 1),
                )
            if ci % 2 == 0:
                nc.vector.tensor_copy(wy_sb[ci][:], wyp[:])
            else:
                nc.scalar.activation(wy_sb[ci][:], wyp[:], AF.Copy)

    if lvl < 5:
        return
    # ================= Phase C: y^T = Wy^T x + b ========================
    with (
        tc.tile_pool(name="ys", bufs=3) as ypool,
        tc.tile_pool(name="yp", bufs=3, space="PSUM") as ypsum,
    ):
        for ns in range(NS):
            nsl = slice(ns * SL, (ns + 1) * SL)
            for co in range(KC):
                yp = ypsum.tile([128, SL], F32, tag="y_ps")
                for ci in range(KC):
                    nc.tensor.matmul(
                        yp[:], wy_sb[ci][:, co * 128:(co + 1) * 128],
                        x_sb[ci][ns][:],
                        start=(ci == 0), stop=(ci == KC - 1),
                    )
                ysb = ypool.tile([128, SL], F32, tag="y_sb")
                nc.scalar.activation(
                    ysb[:], yp[:], AF.Identity,
                    bias=bp_sb[co][:, 0:1], scale=1.0,
                )
                nc.sync.dma_start(yd[co * 128:(co + 1) * 128, nsl], ysb[:])


_NC_CACHE = None


def kernel(x, w_qkv, w_proj, b_proj, num_heads):
    x = np.ascontiguousarray(np.asarray(x, dtype=np.float32))
    w_qkv = np.ascontiguousarray(np.asarray(w_qkv, dtype=np.float32))
    w_proj = np.ascontiguousarray(np.asarray(w_proj, dtype=np.float32))
    b_proj = np.ascontiguousarray(np.asarray(b_proj, dtype=np.float32))
    assert int(num_heads) == NH
    assert x.shape == (B, C, H, W)

    xs = x.reshape(B, C, N)
    bp2 = b_proj.reshape(C, 1)
    in_maps = [
        {"x": xs[b], "w_qkv": w_qkv, "w_proj": w_proj, "b_proj": bp2}
        for b in range(B)
    ]
    global _NC_CACHE
    if _NC_CACHE is None:
        _NC_CACHE = build_nc()
    res = bass_utils.run_bass_kernel_spmd(_NC_CACHE, in_maps, list(range(B)))
    y = np.stack([res.results[b]["y"] for b in range(B)])
    return y.reshape(B, C, H, W).astype(np.float32)


if __name__ == "__main__":
    nc = build_nc()
    n_inst = sum(len(bb.instructions) for bb in nc.main_func.blocks)
    print(f"built OK, {n_inst} instructions")


# revision 17
# speedup vs baseline: 1.3326x; 1.0241x over previous
"""Channel-attention (transposed attention) Trainium2 Bass kernel.

Reference computation (per batch b of 8, one NeuronCore each):
    X    = x[b].reshape(C, N).T                    # [N, C], N = 64*64 = 4096
    qkv  = X @ w_qkv                               # [N, 3C]
    q, k, v : per-head [N, hd], nh=8, hd=64
    logits_h = k_h.T @ v_h                         # [hd, hd]
    attn_h   = softmax(scale * logits_h, axis=-1)  # scale = hd**-0.5 = 1/8
    out_h    = q_h @ attn_h.T                      # [N, hd]
    y[b] = (concat_h(out_h) @ w_proj + b_proj).T   # [C, N]

Sharding: data-parallel over batch, 1 batch item per core, no collectives.

Algebraic restructuring (the whole point of this kernel):

1. Gram trick. logits_h = k_h^T v_h = Wk_h^T (X^T X) Wv_h, so k and v are
   never materialized. G = X^T X is one [C, C] matmul with contraction
   over the 4096 tokens (PE-transposes of x feed it), then
   T = G @ Wv  ([C, C]) and lg = Wk^T T (pair-packed [128,128] blocks)
   are tiny. This kills the [N, 2C] kv projection and its PSUM->SBUF
   copies entirely.

2. Weight folding. out_h = q_h @ (E_h / s_h)^T with E = exp(scale*(lg-max)),
   s = rowsum(E), and y^T = w_proj^T out^T + b. Fold:
       y^T = Wy^T X^T,  Wy = Wq @ M^T,  M^T = blockdiag(E_h)^T-applied
       via  M^T[64h+e, :] = sum_d E_h[d,e] * (w_proj[64h+d, :] / s_h[d])
   M^T is 4 [128,512] matmuls (block-diag exp as lhsT, row-scaled w_proj
   as rhs), Wy is 16 more. This kills the q^T projection ([C, N]) and
   the attention-apply stage: phase C is a single [C, C] x [C, N] GEMM
   reading the resident x tiles directly.

All large matmuls are float32r (fp32 bytes, FP22 multiply): 1 PE
cycle/column at free-dim >= 256 vs 4 for fp32. PE transposes in f32r
run 1.5 cycles/row vs 2.0 for fp32. The softmax itself (reductions,
exp) is exact fp32.

Per-rep PE budget ~210k cycles vs ~396k for the direct formulation.
"""

import numpy as np

import concourse.bass as bass
import concourse.mybir as mybir
import concourse.tile as tile
from concourse import bass_utils

F32 = mybir.dt.float32
F32R = mybir.dt.float32r
AF = mybir.ActivationFunctionType
AX = mybir.AxisListType.X

# Problem shape (hardcoded per contest contract).
B = 8
C = 512
H = W = 64
N = H * W            # 4096 tokens per batch
NH = 8               # heads
HD = C // NH         # 64
SCALE = HD ** -0.5   # 1/8
KC = C // 128        # 4 chunks of 128 channels
NS = 8               # n-slices of 512 tokens
SL = N // NS         # 512
TT = SL // 128       # 4 token tiles of 128 per slice
HP = NH // 2         # 4 head pairs


def _r(ap):
    return ap.bitcast(F32R)


def _f(ap):
    return ap.bitcast(F32)


def _split_multi_waits(nc, max_waits=1):
    """The walrus build in this container encodes at most one sync-wait
    command per instruction (setupSyncWait raises "Too many sync wait
    commands" otherwise — the Tile kernel-tail drain carries several).
    Hoist excess waits onto same-engine NOPs immediately preceding the
    instruction; engine-FIFO order preserves the semantics."""
    n_split = 0
    for bb in nc.main_func.blocks:
        new_insts = []
        for ins in bb.instructions:
            si = ins.sync_info
            waits = list(si.on_wait) if si and si.on_wait else []
            if len(waits) > max_waits:
                extra, keep = waits[:-max_waits], waits[-max_waits:]
                while extra:
                    chunk, extra = extra[:max_waits], extra[max_waits:]
                    nop = mybir.InstNoOp(
                        name=nc.get_next_instruction_name(),
                        ins=[], outs=[],
                        engine=ins.engine,
                        sync_info=mybir.SyncInfo(on_wait=chunk, on_update=[]),
                    )
                    nc.register_instruction(nop)
                    new_insts.append(nop)
                    n_split += 1
                si.on_wait = keep
            new_insts.append(ins)
        bb.instructions[:] = new_insts
    return n_split


def build_nc(reps=1, phases='full'):
    nc = bass.Bass("TRN2", debug=False, num_devices=B)

    x_t = nc.dram_tensor("x", [C, N], F32, kind="ExternalInput")
    wq_t = nc.dram_tensor("w_qkv", [C, 3 * C], F32, kind="ExternalInput")
    wp_t = nc.dram_tensor("w_proj", [C, C], F32, kind="ExternalInput")
    bp_t = nc.dram_tensor("b_proj", [C, 1], F32, kind="ExternalInput")
    y_t = nc.dram_tensor("y", [C, N], F32, kind="ExternalOutput")
    id_t = nc.inline_tensor(np.eye(128, dtype=np.float32), name="id128")

    xd, wqd, wpd, bpd, yd = x_t.ap(), wq_t.ap(), wp_t.ap(), bp_t.ap(), y_t.ap()

    with tile.TileContext(nc) as tc:
        with (
            tc.tile_pool(name="const", bufs=1) as cpool,
            tc.tile_pool(name="xres", bufs=1) as xrpool,
        ):
            # resident x: [C, N] as 4 chunks x 8 slices of [128, 512].
            # slice-0 chunks lead the DMA queue (opening transposes need
            # them), weights follow, slices 1..7 stream per-slice.
            x_sb = [
                [xrpool.tile([128, SL], F32R, name=f"x{k}_{ns}", tag=f"x{k}_{ns}")
                 for ns in range(NS)]
                for k in range(KC)
            ]
            for k in range(KC):
                nc.sync.dma_start(x_sb[k][0][:], _r(xd[k * 128:(k + 1) * 128, 0:SL]))

            wq_sb = [cpool.tile([128, 3 * C], F32R, name=f"wq{k}", tag=f"wq{k}")
                     for k in range(KC)]
            id_sb = cpool.tile([128, 128], F32R, tag="id")
            for k in range(KC):
                r = slice(k * 128, (k + 1) * 128)
                for s in range(3):
                    cs = slice(s * C, (s + 1) * C)
                    nc.sync.dma_start(wq_sb[k][:, cs], _r(wqd[r, cs]))
            nc.sync.dma_start(id_sb[:], _r(id_t.ap()[:, :]))

            wp_sb = [cpool.tile([128, C], F32R, name=f"wp{k}", tag=f"wp{k}")
                     for k in range(KC)]
            bp_sb = [cpool.tile([128, 1], F32, name=f"bp{k}", tag=f"bp{k}")
                     for k in range(KC)]

            # Wq^T, data-independent: 16 PE transposes at startup.
            wqT_sb = [cpool.tile([128, C], F32R, name=f"wqT{kq}", tag=f"wqT{kq}")
                      for kq in range(KC)]
            with tc.tile_pool(name="wtp", bufs=2, space="PSUM") as wtpool:
                for kq in range(KC):
                    tp = wtpool.tile([128, C], F32, tag="wqT_ps")
                    for kci in range(KC):
                        nc.tensor.matmul(
                            _r(tp[:, kci * 128:(kci + 1) * 128]),
                            wq_sb[kci][:, kq * 128:(kq + 1) * 128],
                            id_sb[:], is_transpose=True,
                            start=(kci == 0), stop=(kci == KC - 1),
                        )
                    nc.vector.tensor_copy(wqT_sb[kq][:], tp[:])

            for _rep in range(reps):
                _build_one_pass(nc, tc, cpool, wq_sb, wqT_sb, wp_sb, bp_sb,
                                id_sb, x_sb, xd, yd, wpd, bpd,
                                first_rep=(_rep == 0), phases=phases)
    _split_multi_waits(nc)
    return nc


def _build_one_pass(nc, tc, cpool, wq_sb, wqT_sb, wp_sb, bp_sb, id_sb, x_sb,
                    xd, yd, wpd, bpd, first_rep=True, phases="full"):
    lvl = ["dma", "gram", "logits", "soft", "wy", "full"].index(phases)

    # per-rep work tiles (stable tags: same storage every rep)
    G_sb = [cpool.tile([128, C], F32R, name=f"G{k}", tag=f"G{k}") for k in range(KC)]
    T_sb = [cpool.tile([128, C], F32R, name=f"T{k}", tag=f"T{k}") for k in range(KC)]
    mt_sb = [cpool.tile([128, C], F32R, name=f"mt{p}", tag=f"mt{p}") for p in range(HP)]
    wy_sb = [cpool.tile([128, C], F32R, name=f"wy{k}", tag=f"wy{k}") for k in range(KC)]
    bd = [cpool.tile([128, 128], F32, name=f"bd{p}", tag=f"bd{p}") for p in range(HP)]
    bd2 = [cpool.tile([128, 128], F32R, name=f"bd2{p}", tag=f"bd2{p}") for p in range(HP)]
    mx = cpool.tile([128, HP], F32, name="mx", tag="mx")
    bias = cpool.tile([128, HP], F32, name="bias", tag="bias")
    ssum = cpool.tile([128, HP], F32, name="ssum", tag="ssum")
    recip = cpool.tile([128, HP], F32, name="recip", tag="recip")

    # ================= Phase A: transpose x + Gram accumulation =========
    with (
        tc.tile_pool(name="xt", bufs=3) as xtpool,
        tc.tile_pool(name="gps", bufs=1, space="PSUM") as gpool,
        tc.tile_pool(name="tps", bufs=2, space="PSUM") as tpool,
    ):
        G_ps = [gpool.tile([128, C], F32, name=f"g_ps{k}", tag=f"g_ps{k}")
                for k in range(KC)]
        for ns in range(NS):
            if not (first_rep and ns == 0):
                nsl = slice(ns * SL, (ns + 1) * SL)
                for k in range(KC):
                    nc.sync.dma_start(
                        x_sb[k][ns][:], _r(xd[k * 128:(k + 1) * 128, nsl])
                    )
            if lvl < 1:
                continue
            for t in range(TT):
                tsl = slice(t * 128, (t + 1) * 128)
                tp = tpool.tile([128, C], F32, tag="xt_ps")
                for k in range(KC):
                    nc.tensor.matmul(
                        _r(tp[:, k * 128:(k + 1) * 128]),
                        x_sb[k][ns][:, tsl], id_sb[:], is_transpose=True,
                        start=(k == 0), stop=(k == KC - 1),
                    )
                xt = xtpool.tile([128, C], F32R, tag="xt_sb")
                # alternate copy engines so neither DVE nor ACT gates the PE
                if t % 2 == 0:
                    nc.vector.tensor_copy(xt[:], tp[:])
                else:
                    nc.scalar.activation(xt[:], tp[:], AF.Copy)
                for k in range(KC):
                    nc.tensor.matmul(
                        G_ps[k][:], xt[:, k * 128:(k + 1) * 128], xt[:],
                        start=(ns == 0 and t == 0),
                        stop=(ns == NS - 1 and t == TT - 1),
                    )
        if lvl >= 1:
            for k in range(KC):
                if k % 2 == 0:
                    nc.vector.tensor_copy(G_sb[k][:], G_ps[k][:])
                else:
                    nc.scalar.activation(G_sb[k][:], G_ps[k][:], AF.Copy)

    # deferred weight loads: w_proj/b_proj first needed in phase B tail
    if first_rep:
        for k in range(KC):
            r = slice(k * 128, (k + 1) * 128)
            nc.sync.dma_start(wp_sb[k][:], _r(wpd[r, :]))
            nc.sync.dma_start(bp_sb[k][:], bpd[r, :])

    if lvl < 2:
        return
    # ================= Phase B: T = G Wv, logits, softmax, M^T, Wy ======
    with tc.tile_pool(name="Tps", bufs=1, space="PSUM") as Tpool:
        for kc in range(KC):
            Tp = Tpool.tile([128, C], F32, tag=f"T_ps{kc}")
            for k2 in range(KC):
                nc.tensor.matmul(
                    Tp[:], G_sb[k2][:, kc * 128:(kc + 1) * 128],
                    wq_sb[k2][:, 2 * C:3 * C],
                    start=(k2 == 0), stop=(k2 == KC - 1),
                )
            if kc % 2 == 0:
                nc.vector.tensor_copy(T_sb[kc][:], Tp[:])
            else:
                nc.scalar.activation(T_sb[kc][:], Tp[:], AF.Copy)

    with tc.tile_pool(name="lgp", bufs=1, space="PSUM") as lgpool:
        # per-pair lg banks [128, 512] at f32r full rate (free-dim 512,
        # 1 cyc/col): the matmul computes pair-p's d rows against ALL
        # 512 v-columns; only head h's own 64-col block is ever read
        # (cols 64h), the rest is junk. Same cycles as the exact
        # 128-free variant would take at 4 cyc/col, but pairs complete
        # (stop=) individually so the softmax pipelines per pair.
        lgp = [lgpool.tile([128, C], F32, name=f"lg{p}", tag=f"lg{p}")
               for p in range(HP)]
        for p in range(HP):
            for kc in range(KC):
                nc.tensor.matmul(
                    lgp[p][:],
                    wq_sb[kc][:, C + p * 128:C + (p + 1) * 128],
                    T_sb[kc][:],
                    start=(kc == 0), stop=(kc == KC - 1),
                )
        if lvl < 3:
            return
        # softmax over each head's diag block, exp into block-diag bd[p]
        if first_rep:
            for p in range(HP):
                nc.gpsimd.memset(bd[p][:], 0.0)
        for p in range(HP):
            for par in range(2):
                psl = slice(64 * par, 64 * par + 64)
                csl = slice((2 * p + par) * 64, (2 * p + par) * 64 + 64)
                nc.vector.reduce_max(mx[psl, p:p + 1], lgp[p][psl, csl], axis=AX)
            nc.vector.tensor_scalar_mul(bias[:, p:p + 1], mx[:, p:p + 1], -SCALE)
            for par in range(2):
                psl = slice(64 * par, 64 * par + 64)
                csl = slice((2 * p + par) * 64, (2 * p + par) * 64 + 64)
                nc.scalar.activation(
                    bd[p][psl, psl], lgp[p][psl, csl], AF.Exp,
                    bias=bias[psl, p:p + 1], scale=SCALE,
                )
                nc.vector.reduce_sum(ssum[psl, p:p + 1], bd[p][psl, psl], axis=AX)
            nc.vector.reciprocal(recip[:, p:p + 1], ssum[:, p:p + 1])
            # fold 1/rowsum into the tiny exp matrix (rows d of head
            # 2p+par scaled by recip), not into the [128,512] w_proj
            nc.vector.tensor_scalar_mul(bd2[p][:], bd[p][:], recip[:, p:p + 1])

    if lvl < 4:
        return
    # M^T[128p + 64par + e, c] = sum_d (E/s)[d,e] * wp[128p + 64par + d, c]
    # then immediately fold pair p into Wy[ci, co] = sum_q Wq[ci,q] M^T[q,co]
    with (
        tc.tile_pool(name="mps", bufs=2, space="PSUM") as mpool,
        tc.tile_pool(name="wyp", bufs=1, space="PSUM") as wypool,
    ):
        wyps = [wypool.tile([128, C], F32, name=f"wy_ps{ci}", tag=f"wy_ps{ci}")
                for ci in range(KC)]
        for p in range(HP):
            mp = mpool.tile([128, C], F32, tag="mt_ps")
            nc.tensor.matmul(mp[:], bd2[p][:], wp_sb[p][:], start=True, stop=True)
            if p % 2 == 0:
                nc.vector.tensor_copy(mt_sb[p][:], mp[:])
            else:
                nc.scalar.activation(mt_sb[p][:], mp[:], AF.Copy)
            for ci in range(KC):
                nc.tensor.matmul(
                    wyps[ci][:], wqT_sb[p][:, ci * 128:(ci + 1) * 128], mt_sb[p][:],
                    start=(p == 0), stop=(p == HP - 1),
                )
        for ci in range(KC):
            if ci % 2 == 0:
                nc.vector.tensor_copy(wy_sb[ci][:], wyps[ci][:])
            else:
                nc.scalar.activation(wy_sb[ci][:], wyps[ci][:], AF.Copy)

    if lvl < 5:
        return
    # ================= Phase C: y^T = Wy^T x + b ========================
    with (
        tc.tile_pool(name="ys", bufs=3) as ypool,
        tc.tile_pool(name="yp", bufs=3, space="PSUM") as ypsum,
    ):
        for ns in range(NS):
            nsl = slice(ns * SL, (ns + 1) * SL)
            for co in range(KC):
                yp = ypsum.tile([128, SL], F32, tag="y_ps")
                for ci in range(KC):
                    nc.tensor.matmul(
                        yp[:], wy_sb[ci][:, co * 128:(co + 1) * 128],
                        x_sb[ci][ns][:],
                        start=(ci == 0), stop=(ci == KC - 1),
                    )
                ysb = ypool.tile([128, SL], F32, tag="y_sb")
                nc.scalar.activation(
                    ysb[:], yp[:], AF.Identity,
                    bias=bp_sb[co][:, 0:1], scale=1.0,
                )
                nc.sync.dma_start(yd[co * 128:(co + 1) * 128, nsl], ysb[:])


_NC_CACHE = None


def kernel(x, w_qkv, w_proj, b_proj, num_heads):
    x = np.ascontiguousarray(np.asarray(x, dtype=np.float32))
    w_qkv = np.ascontiguousarray(np.asarray(w_qkv, dtype=np.float32))
    w_proj = np.ascontiguousarray(np.asarray(w_proj, dtype=np.float32))
    b_proj = np.ascontiguousarray(np.asarray(b_proj, dtype=np.float32))
    assert int(num_heads) == NH
    assert x.shape == (B, C, H, W)

    xs = x.reshape(B, C, N)
    bp2 = b_proj.reshape(C, 1)
    in_maps = [
        {"x": xs[b], "w_qkv": w_qkv, "w_proj": w_proj, "b_proj": bp2}
        for b in range(B)
    ]
    global _NC_CACHE
    if _NC_CACHE is None:
        _NC_CACHE = build_nc()
    res = bass_utils.run_bass_kernel_spmd(_NC_CACHE, in_maps, list(range(B)))
    y = np.stack([res.results[b]["y"] for b in range(B)])
    return y.reshape(B, C, H, W).astype(np.float32)


if __name__ == "__main__":
    nc = build_nc()
    n_inst = sum(len(bb.instructions) for bb in nc.main_func.blocks)
    print(f"built OK, {n_inst} instructions")


# revision 21
# speedup vs baseline: 1.3412x; 1.0064x over previous
"""Channel-attention (transposed attention) Trainium2 Bass kernel.

Reference computation (per batch b of 8, one NeuronCore each):
    X    = x[b].reshape(C, N).T                    # [N, C], N = 64*64 = 4096
    qkv  = X @ w_qkv                               # [N, 3C]
    q, k, v : per-head [N, hd], nh=8, hd=64
    logits_h = k_h.T @ v_h                         # [hd, hd]
    attn_h   = softmax(scale * logits_h, axis=-1)  # scale = hd**-0.5 = 1/8
    out_h    = q_h @ attn_h.T                      # [N, hd]
    y[b] = (concat_h(out_h) @ w_proj + b_proj).T   # [C, N]

Sharding: data-parallel over batch, 1 batch item per core, no collectives.

Algebraic restructuring (the whole point of this kernel):

1. Gram trick. logits_h = k_h^T v_h = Wk_h^T (X^T X) Wv_h, so k and v are
   never materialized. G = X^T X is one [C, C] matmul with contraction
   over the 4096 tokens (PE-transposes of x feed it), then
   T = G @ Wv  ([C, C]) and lg = Wk^T T (pair-packed [128,128] blocks)
   are tiny. This kills the [N, 2C] kv projection and its PSUM->SBUF
   copies entirely.

2. Weight folding. out_h = q_h @ (E_h / s_h)^T with E = exp(scale*(lg-max)),
   s = rowsum(E), and y^T = w_proj^T out^T + b. Fold:
       y^T = Wy^T X^T,  Wy = Wq @ M^T,  M^T = blockdiag(E_h)^T-applied
       via  M^T[64h+e, :] = sum_d E_h[d,e] * (w_proj[64h+d, :] / s_h[d])
   M^T is 4 [128,512] matmuls (block-diag exp as lhsT, row-scaled w_proj
   as rhs), Wy is 16 more. This kills the q^T projection ([C, N]) and
   the attention-apply stage: phase C is a single [C, C] x [C, N] GEMM
   reading the resident x tiles directly.

All large matmuls are float32r (fp32 bytes, FP22 multiply): 1 PE
cycle/column at free-dim >= 256 vs 4 for fp32. PE transposes in f32r
run 1.5 cycles/row vs 2.0 for fp32. The softmax itself (reductions,
exp) is exact fp32.

Per-rep PE budget ~210k cycles vs ~396k for the direct formulation.
"""

import numpy as np

import concourse.bass as bass
import concourse.mybir as mybir
import concourse.tile as tile
from concourse import bass_utils

F32 = mybir.dt.float32
F32R = mybir.dt.float32r
AF = mybir.ActivationFunctionType
AX = mybir.AxisListType.X

# Problem shape (hardcoded per contest contract).
B = 8
C = 512
H = W = 64
N = H * W            # 4096 tokens per batch
NH = 8               # heads
HD = C // NH         # 64
SCALE = HD ** -0.5   # 1/8
KC = C // 128        # 4 chunks of 128 channels
NS = 8               # n-slices of 512 tokens
SL = N // NS         # 512
TT = SL // 128       # 4 token tiles of 128 per slice
HP = NH // 2         # 4 head pairs


def _r(ap):
    return ap.bitcast(F32R)


def _f(ap):
    return ap.bitcast(F32)


def _split_multi_waits(nc, max_waits=1):
    """The walrus build in this container encodes at most one sync-wait
    command per instruction (setupSyncWait raises "Too many sync wait
    commands" otherwise — the Tile kernel-tail drain carries several).
    Hoist excess waits onto same-engine NOPs immediately preceding the
    instruction; engine-FIFO order preserves the semantics."""
    n_split = 0
    for bb in nc.main_func.blocks:
        new_insts = []
        for ins in bb.instructions:
            si = ins.sync_info
            waits = list(si.on_wait) if si and si.on_wait else []
            if len(waits) > max_waits:
                extra, keep = waits[:-max_waits], waits[-max_waits:]
                while extra:
                    chunk, extra = extra[:max_waits], extra[max_waits:]
                    nop = mybir.InstNoOp(
                        name=nc.get_next_instruction_name(),
                        ins=[], outs=[],
                        engine=ins.engine,
                        sync_info=mybir.SyncInfo(on_wait=chunk, on_update=[]),
                    )
                    nc.register_instruction(nop)
                    new_insts.append(nop)
                    n_split += 1
                si.on_wait = keep
            new_insts.append(ins)
        bb.instructions[:] = new_insts
    return n_split


def build_nc(reps=1, phases='full'):
    nc = bass.Bass("TRN2", debug=False, num_devices=B)

    x_t = nc.dram_tensor("x", [C, N], F32, kind="ExternalInput")
    wq_t = nc.dram_tensor("w_qkv", [C, 3 * C], F32, kind="ExternalInput")
    wp_t = nc.dram_tensor("w_proj", [C, C], F32, kind="ExternalInput")
    bp_t = nc.dram_tensor("b_proj", [C, 1], F32, kind="ExternalInput")
    y_t = nc.dram_tensor("y", [C, N], F32, kind="ExternalOutput")
    id_t = nc.inline_tensor(np.eye(128, dtype=np.float32), name="id128")

    xd, wqd, wpd, bpd, yd = x_t.ap(), wq_t.ap(), wp_t.ap(), bp_t.ap(), y_t.ap()

    with tile.TileContext(nc) as tc:
        with (
            tc.tile_pool(name="const", bufs=1) as cpool,
            tc.tile_pool(name="xres", bufs=1) as xrpool,
        ):
            # resident x: [C, N] as 4 chunks x 8 slices of [128, 512].
            # slice-0 chunks lead the DMA queue (opening transposes need
            # them), weights follow, slices 1..7 stream per-slice.
            x_sb = [
                [xrpool.tile([128, SL], F32R, name=f"x{k}_{ns}", tag=f"x{k}_{ns}")
                 for ns in range(NS)]
                for k in range(KC)
            ]
            for k in range(KC):
                nc.sync.dma_start(x_sb[k][0][:], _r(xd[k * 128:(k + 1) * 128, 0:SL]))

            wq_sb = [cpool.tile([128, 3 * C], F32R, name=f"wq{k}", tag=f"wq{k}")
                     for k in range(KC)]
            id_sb = cpool.tile([128, 128], F32R, tag="id")
            for k in range(KC):
                r = slice(k * 128, (k + 1) * 128)
                for s in range(3):
                    cs = slice(s * C, (s + 1) * C)
                    nc.sync.dma_start(wq_sb[k][:, cs], _r(wqd[r, cs]))
            nc.sync.dma_start(id_sb[:], _r(id_t.ap()[:, :]))

            wp_sb = [cpool.tile([128, C], F32R, name=f"wp{k}", tag=f"wp{k}")
                     for k in range(KC)]
            bp_sb = [cpool.tile([128, 1], F32, name=f"bp{k}", tag=f"bp{k}")
                     for k in range(KC)]

            # Explicit PSUM choreography: 8 named banks, assigned so that
            # phase C (banks 6,7) of rep r is disjoint from phase A
            # (banks 0-5) of rep r+1 — the cross-rep overlap that hides
            # the next rep's transpose+Gram work under the projection.
            with tc.tile_pool(name="ps", bufs=1, space="PSUM") as pspool:
                bank = [pspool.tile([128, C], F32, name=f"bank{i}", tag=f"bank{i}")
                        for i in range(8)]

                # Wq^T, data-independent: 16 PE transposes at startup.
                wqT_sb = [cpool.tile([128, C], F32R, name=f"wqT{kq}", tag=f"wqT{kq}")
                          for kq in range(KC)]
                for kq in range(KC):
                    tp = bank[6 + kq % 2]
                    for kci in range(KC):
                        nc.tensor.matmul(
                            _r(tp[:, kci * 128:(kci + 1) * 128]),
                            wq_sb[kci][:, kq * 128:(kq + 1) * 128],
                            id_sb[:], is_transpose=True,
                            start=(kci == 0), stop=(kci == KC - 1),
                        )
                    nc.vector.tensor_copy(wqT_sb[kq][:], tp[:])

                for _rep in range(reps):
                    _build_one_pass(nc, tc, cpool, wq_sb, wqT_sb, wp_sb, bp_sb,
                                    id_sb, x_sb, xd, yd, wpd, bpd, bank,
                                    first_rep=(_rep == 0), phases=phases)
    _split_multi_waits(nc)
    return nc


def _build_one_pass(nc, tc, cpool, wq_sb, wqT_sb, wp_sb, bp_sb, id_sb, x_sb,
                    xd, yd, wpd, bpd, bank, first_rep=True, phases="full"):
    lvl = ["dma", "gram", "logits", "soft", "wy", "full"].index(phases)

    # per-rep work tiles (stable tags: same storage every rep)
    G_sb = [cpool.tile([128, C], F32R, name=f"G{k}", tag=f"G{k}") for k in range(KC)]
    T_sb = [cpool.tile([128, C], F32R, name=f"T{k}", tag=f"T{k}") for k in range(KC)]
    mt_sb = [cpool.tile([128, C], F32R, name=f"mt{p}", tag=f"mt{p}") for p in range(HP)]
    wy_sb = [cpool.tile([128, C], F32R, name=f"wy{k}", tag=f"wy{k}") for k in range(KC)]
    bd = [cpool.tile([128, 128], F32, name=f"bd{p}", tag=f"bd{p}") for p in range(HP)]
    bd2 = [cpool.tile([128, 128], F32R, name=f"bd2{p}", tag=f"bd2{p}") for p in range(HP)]
    mx = cpool.tile([128, HP], F32, name="mx", tag="mx")
    bias = cpool.tile([128, HP], F32, name="bias", tag="bias")
    ssum = cpool.tile([128, HP], F32, name="ssum", tag="ssum")
    recip = cpool.tile([128, HP], F32, name="recip", tag="recip")

    # ================= Phase A: transpose x + Gram accumulation =========
    # banks 0-3: G accumulators; banks 4,5: transpose staging. Disjoint
    # from the previous rep's phase C (banks 6,7), so this phase runs
    # under it. x loads go on the Activation DGE queue so they are not
    # stuck behind the previous rep's y stores on the sync queue.
    G_ps = bank[0:KC]
    with tc.tile_pool(name="xt", bufs=4) as xtpool:
        for ns in range(NS):
            if not (first_rep and ns == 0):
                nsl = slice(ns * SL, (ns + 1) * SL)
                for k in range(KC):
                    nc.scalar.dma_start(
                        x_sb[k][ns][:], _r(xd[k * 128:(k + 1) * 128, nsl])
                    )
            if lvl < 1:
                continue
            for t in range(TT):
                tsl = slice(t * 128, (t + 1) * 128)
                tp = bank[4 + t % 2]
                for k in range(KC):
                    nc.tensor.matmul(
                        _r(tp[:, k * 128:(k + 1) * 128]),
                        x_sb[k][ns][:, tsl], id_sb[:], is_transpose=True,
                        start=(k == 0), stop=(k == KC - 1),
                    )
                xt = xtpool.tile([128, C], F32R, tag="xt_sb")
                # alternate copy engines so neither DVE nor ACT gates the PE
                if t % 2 == 0:
                    nc.vector.tensor_copy(xt[:], tp[:])
                else:
                    nc.scalar.activation(xt[:], tp[:], AF.Copy)
                for k in range(KC):
                    nc.tensor.matmul(
                        G_ps[k][:], xt[:, k * 128:(k + 1) * 128], xt[:],
                        start=(ns == 0 and t == 0),
                        stop=(ns == NS - 1 and t == TT - 1),
                    )
        if lvl >= 1:
            for k in range(KC):
                if k % 2 == 0:
                    nc.vector.tensor_copy(G_sb[k][:], G_ps[k][:])
                else:
                    nc.scalar.activation(G_sb[k][:], G_ps[k][:], AF.Copy)

    # deferred weight loads: w_proj/b_proj first needed in phase B tail
    if first_rep:
        for k in range(KC):
            r = slice(k * 128, (k + 1) * 128)
            nc.sync.dma_start(wp_sb[k][:], _r(wpd[r, :]))
            nc.sync.dma_start(bp_sb[k][:], bpd[r, :])

    if lvl < 2:
        return
    # ================= Phase B: T = G Wv, logits, softmax, M^T, Wy ======
    for kc in range(KC):
        Tp = bank[6 + kc % 2]
        for k2 in range(KC):
            nc.tensor.matmul(
                Tp[:], G_sb[k2][:, kc * 128:(kc + 1) * 128],
                wq_sb[k2][:, 2 * C:3 * C],
                start=(k2 == 0), stop=(k2 == KC - 1),
            )
        if kc % 2 == 0:
            nc.vector.tensor_copy(T_sb[kc][:], Tp[:])
        else:
            nc.scalar.activation(T_sb[kc][:], Tp[:], AF.Copy)

    # per-pair lg banks [128, 512] at f32r full rate (free-dim 512,
    # 1 cyc/col): the matmul computes pair-p's d rows against ALL
    # 512 v-columns; only head h's own 64-col block is ever read
    # (cols 64h), the rest is junk. Same cycles as the exact
    # 128-free variant would take at 4 cyc/col, but pairs complete
    # (stop=) individually so the softmax pipelines per pair.
    lgp = bank[0:HP]
    for p in range(HP):
        for kc in range(KC):
            nc.tensor.matmul(
                lgp[p][:],
                wq_sb[kc][:, C + p * 128:C + (p + 1) * 128],
                T_sb[kc][:],
                start=(kc == 0), stop=(kc == KC - 1),
            )
    if lvl < 3:
        return
    # softmax over each head's diag block, exp into block-diag bd[p]
    if first_rep:
        for p in range(HP):
            nc.gpsimd.memset(bd[p][:], 0.0)
    for p in range(HP):
        for par in range(2):
            psl = slice(64 * par, 64 * par + 64)
            csl = slice((2 * p + par) * 64, (2 * p + par) * 64 + 64)
            nc.vector.reduce_max(mx[psl, p:p + 1], lgp[p][psl, csl], axis=AX)
        nc.vector.tensor_scalar_mul(bias[:, p:p + 1], mx[:, p:p + 1], -SCALE)
        for par in range(2):
            psl = slice(64 * par, 64 * par + 64)
            csl = slice((2 * p + par) * 64, (2 * p + par) * 64 + 64)
            nc.scalar.activation(
                bd[p][psl, psl], lgp[p][psl, csl], AF.Exp,
                bias=bias[psl, p:p + 1], scale=SCALE,
            )
            nc.vector.reduce_sum(ssum[psl, p:p + 1], bd[p][psl, psl], axis=AX)
        nc.vector.reciprocal(recip[:, p:p + 1], ssum[:, p:p + 1])
        # fold 1/rowsum into the tiny exp matrix (rows d of head
        # 2p+par scaled by recip), not into the [128,512] w_proj
        nc.vector.tensor_scalar_mul(bd2[p][:], bd[p][:], recip[:, p:p + 1])

    if lvl < 4:
        return
    # M^T[128p + 64par + e, c] = sum_d (E/s)[d,e] * wp[128p + 64par + d, c]
    # then immediately fold pair p into Wy[ci, co] = sum_q Wq[ci,q] M^T[q,co]
    wyps = bank[0:KC]
    for p in range(HP):
        mp = bank[4 + p % 2]
        nc.tensor.matmul(mp[:], bd2[p][:], wp_sb[p][:], start=True, stop=True)
        if p % 2 == 0:
            nc.vector.tensor_copy(mt_sb[p][:], mp[:])
        else:
            nc.scalar.activation(mt_sb[p][:], mp[:], AF.Copy)
        for ci in range(KC):
            nc.tensor.matmul(
                wyps[ci][:], wqT_sb[p][:, ci * 128:(ci + 1) * 128], mt_sb[p][:],
                start=(p == 0), stop=(p == HP - 1),
            )
    for ci in range(KC):
        if ci % 2 == 0:
            nc.vector.tensor_copy(wy_sb[ci][:], wyps[ci][:])
        else:
            nc.scalar.activation(wy_sb[ci][:], wyps[ci][:], AF.Copy)

    if lvl < 5:
        return
    # ================= Phase C: y^T = Wy^T x + b ========================
    # banks 6,7 only: disjoint from the next rep's phase A banks.
    with tc.tile_pool(name="ys", bufs=4) as ypool:
        for ns in range(NS):
            nsl = slice(ns * SL, (ns + 1) * SL)
            for co in range(KC):
                yp = bank[6 + (ns * KC + co) % 2]
                for ci in range(KC):
                    nc.tensor.matmul(
                        yp[:], wy_sb[ci][:, co * 128:(co + 1) * 128],
                        x_sb[ci][ns][:],
                        start=(ci == 0), stop=(ci == KC - 1),
                    )
                ysb = ypool.tile([128, SL], F32, tag="y_sb")
                nc.scalar.activation(
                    ysb[:], yp[:], AF.Identity,
                    bias=bp_sb[co][:, 0:1], scale=1.0,
                )
                nc.sync.dma_start(yd[co * 128:(co + 1) * 128, nsl], ysb[:])


_NC_CACHE = None


def kernel(x, w_qkv, w_proj, b_proj, num_heads):
    x = np.ascontiguousarray(np.asarray(x, dtype=np.float32))
    w_qkv = np.ascontiguousarray(np.asarray(w_qkv, dtype=np.float32))
    w_proj = np.ascontiguousarray(np.asarray(w_proj, dtype=np.float32))
    b_proj = np.ascontiguousarray(np.asarray(b_proj, dtype=np.float32))
    assert int(num_heads) == NH
    assert x.shape == (B, C, H, W)

    xs = x.reshape(B, C, N)
    bp2 = b_proj.reshape(C, 1)
    in_maps = [
        {"x": xs[b], "w_qkv": w_qkv, "w_proj": w_proj, "b_proj": bp2}
        for b in range(B)
    ]
    global _NC_CACHE
    if _NC_CACHE is None:
        _NC_CACHE = build_nc()
    res = bass_utils.run_bass_kernel_spmd(_NC_CACHE, in_maps, list(range(B)))
    y = np.stack([res.results[b]["y"] for b in range(B)])
    return y.reshape(B, C, H, W).astype(np.float32)


if __name__ == "__main__":
    nc = build_nc()
    n_inst = sum(len(bb.instructions) for bb in nc.main_func.blocks)
    print(f"built OK, {n_inst} instructions")
